# revision 26
# baseline (speedup 1.0000x reference)
"""Fused causal multi-head attention block on 8 Trainium2 NeuronCores.

Problem (GPT-2 style attention, B=2, S=2048, D=1024, H=16, hd=64):
    qkv = x @ w_attn + b_attn ; split q,k,v ; per-head causal softmax(q k^T / 8) v
    out = attn_out @ w_proj + b_proj

Sharding: data parallel on batch (2) x tensor parallel on heads (4 groups of 4
heads). Core c -> batch c//4, head group c%4. Each core computes a partial
[S, D] output (its heads' slice of w_proj rows); host sums the 4 partials per
batch and adds b_proj.

Per-core kernel layout tricks:
- scores are computed TRANSPOSED (scoresT[key, query]) so the softmax
  denominator falls out of the attn@v matmul by appending a ones-column to v:
  [v | 1]^T @ exp(scoresT) yields the unnormalized output and the per-query
  denominator in one PSUM accumulation.
- matmul inputs are fp16 (full PE rate + fast weight loads); all accumulation
  is fp32 in PSUM. exp(s/8) is in [0, ~13], well inside fp16 range.
- causal masking: fully-masked blocks are skipped via restricted matmul
  widths; diagonal blocks get a -30000 triangle accumulated into the score
  PSUM by an identity matmul, so exp() yields exact zeros and the vector
  engine stays out of the score->attnv chain.
- emission is chunk-pipelined (QKV chunk c, attention chunk c, projection
  chunk c) so the PE always has dense matmul work while ScalarE runs exp.
"""

import sys

sys.path.insert(0, "/opt/trn_rl_repo")

import numpy as np

import concourse.bass as bass
import concourse.mybir as mybir
import concourse.tile as tile
from concourse import bacc
from concourse.bass_utils import run_bass_kernel_spmd

F32 = mybir.dt.float32
F16 = mybir.dt.float16
AFT = mybir.ActivationFunctionType

B, S, D, H, HD = 2, 2048, 1024, 16, 64
NCORES = 8
HPC = 4            # heads per core
CH = HPC * HD      # 256 channels per core
VW = HD + 1        # v width incl. ones column
P = 128
KT = D // P        # 8 contraction tiles over D
SQ = 512           # query/N chunk
NSQ = S // SQ      # 4
NST = S // P       # 16 seq tiles
SCALE = 1.0 / np.sqrt(HD)
MASKNEG = -30000.0


def emit_kernel(nc, tc, ap):
    """Emit the per-core program. `ap` is a dict of DRAM APs."""
    with (
        tc.tile_pool(name="const", bufs=1) as cp,
        tc.tile_pool(name="xw", bufs=1) as xw,
        tc.tile_pool(name="act", bufs=1) as acts,
        tc.tile_pool(name="ex", bufs=16) as exp_pool,
        tc.tile_pool(name="dh", bufs=4) as dh_pool,
        tc.tile_pool(name="rc", bufs=2) as rc_pool,
        tc.tile_pool(name="osb", bufs=3) as osb,
        tc.tile_pool(name="psA", bufs=2, space="PSUM") as psA,
        tc.tile_pool(name="psB", bufs=2, space="PSUM") as psB,
        tc.tile_pool(name="psC", bufs=2, space="PSUM") as psC,
    ):
        # ---- PE warmup: dense dummy matmuls while input DMAs stream in.
        # The PE clock-gate (HAM) unthrottles 1.2->2.4 GHz only after ~3.4us
        # of sustained matmul activity; burn that in on scratch data.
        wsrc = cp.tile([P, SQ], F16, name="wsrc", tag="wsrc")
        nc.gpsimd.memset(wsrc, 0.0)
        wps = psB.tile([P, SQ], F32, name="wps", tag="acc")
        for i in range(10):
            nc.tensor.matmul(
                wps, wsrc[:, 0:P], wsrc, start=(i == 0), stop=(i == 9),
            )

        # ---- constants ----
        tri = cp.tile([P, P], F16, name="tri", tag="tri")
        nc.sync.dma_start(tri, ap["tri"])
        bq = cp.tile([P, 2], F32, name="bq", tag="bq")
        nc.sync.dma_start(bq, ap["bq"])
        bk = cp.tile([P, 2], F32, name="bk", tag="bk")
        nc.sync.dma_start(bk, ap["bk"])
        bv = cp.tile([1, HPC * VW], F16, name="bv", tag="bv")
        nc.sync.dma_start(bv, ap["bv"])
        ones1 = cp.tile([1, P], F16, name="ones1", tag="ones1")
        nc.sync.dma_start(ones1, ap["ones1"])

        # ---- weight/x loads (k-tile order so QKV can start early) ----
        xts, wq_t, wk_t, wv_t = [], [], [], []
        for k in range(KT):
            xt = xw.tile([P, S], F16, name=f"xt{k}", tag=f"xt{k}")
            nc.sync.dma_start(xt, ap["xT"][k * P:(k + 1) * P, :])
            xts.append(xt)
            w = xw.tile([P, CH], F16, name=f"wq{k}", tag=f"wq{k}")
            nc.sync.dma_start(w, ap["wq"][k * P:(k + 1) * P, :])
            wq_t.append(w)
            w = xw.tile([P, CH], F16, name=f"wk{k}", tag=f"wk{k}")
            nc.sync.dma_start(w, ap["wk"][k * P:(k + 1) * P, :])
            wk_t.append(w)
            w = xw.tile([P, HPC * VW], F16, name=f"wv{k}", tag=f"wv{k}")
            nc.sync.dma_start(w, ap["wv"][k * P:(k + 1) * P, :])
            wv_t.append(w)
        wp_t = []
        for k in range(2):
            w = xw.tile([P, D], F16, name=f"wp{k}", tag=f"wp{k}")
            nc.sync.dma_start(w, ap["wp"][k * P:(k + 1) * P, :])
            wp_t.append(w)

        # ---- activations living across phases ----
        qT = [acts.tile([P, S], F16, name=f"qT{i}", tag=f"qT{i}") for i in range(2)]
        kTt = [acts.tile([P, S], F16, name=f"kT{i}", tag=f"kT{i}") for i in range(2)]
        vv = acts.tile([P, NST, HPC * VW], F16, name="vv", tag="vv")
        outT = [acts.tile([P, S], F16, name=f"oT{i}", tag=f"oT{i}") for i in range(2)]

        def qkv_qk_group(c, dst, wt, bias, i):
            ps = psA.tile([P, SQ], F32, name="ps", tag="ps")
            for k in range(KT):
                nc.tensor.matmul(
                    ps,
                    wt[k][:, i * P:(i + 1) * P],
                    xts[k][:, c * SQ:(c + 1) * SQ],
                    start=(k == 0),
                    stop=(k == KT - 1),
                )
            with nc.allow_low_precision(reason="fp16 matmul inputs"):
                nc.vector.tensor_scalar_add(
                    dst[i][:, c * SQ:(c + 1) * SQ], ps, bias[:, i:i + 1],
                )

        def qkv_v_group(st):
            # v rows (natural layout + interleaved ones cols)
            ps = psA.tile([P, SQ], F32, name="psv", tag="ps")
            psv = ps[:, 0:HPC * VW]
            for k in range(KT):
                nc.tensor.matmul(
                    psv,
                    xts[k][:, st * P:(st + 1) * P],
                    wv_t[k],
                    start=(k == 0),
                    stop=False,
                )
            # += ones_col(seq) x (bv | interleaved 1.0): v-bias + ones col
            nc.tensor.matmul(psv, ones1, bv, start=False, stop=True)
            with nc.allow_low_precision(reason="fp16 matmul inputs"):
                nc.vector.tensor_copy(vv[:, st, :], psv)

        def qkv_groups(c):
            for dst, wt, bias in ((qT, wq_t, bq), (kTt, wk_t, bk)):
                for i in range(2):
                    yield lambda dst=dst, wt=wt, bias=bias, i=i: \
                        qkv_qk_group(c, dst, wt, bias, i)
            for st in range(4 * c, 4 * c + 4):
                yield lambda st=st: qkv_v_group(st)

        def attention_pair(i, c, fillers=()):
            """Heads 2i (kT/qT rows 0:64) and 2i+1 (rows 64:128) together.

            Both heads' scores for a key tile land in one 2-bank PSUM tile so
            a single exp instruction covers them (halves ScalarE instruction
            count). All scores are emitted before all attnv matmuls: the PE
            stream is in-order, so this keeps the PE on scores while
            ScalarE's exps pipeline behind."""
            nkt = 4 * (c + 1)
            accs = [psB.tile([VW, SQ], F32, name="acc", tag="acc")
                    for _ in range(2)]

            exs = []
            for kt in range(nkt):
                colo = max(0, kt * P - c * SQ)
                diag = colo > 0 or kt * P == c * SQ
                sc2 = psC.tile([P, 2, SQ], F32, name="sc2", tag="sc")
                for j in range(2):
                    ro = j * 64
                    nc.tensor.matmul(
                        sc2[:, j, colo:SQ],
                        kTt[i][ro:ro + 64, kt * P:(kt + 1) * P],
                        qT[i][ro:ro + 64, c * SQ + colo:(c + 1) * SQ],
                        start=True,
                        stop=True,
                    )
                ex2 = exp_pool.tile([P, 2, SQ], F16, name="ex2", tag="ex")
                nc.scalar.activation(
                    ex2[:, :, colo:SQ], sc2[:, :, colo:SQ], AFT.Exp, scale=SCALE,
                )
                if diag:
                    # zero the masked triangle of the diagonal block; runs on
                    # VectorE well before the (much later) attnv consumers
                    nc.vector.tensor_mul(
                        ex2[:, :, colo:colo + P],
                        ex2[:, :, colo:colo + P],
                        tri[:, None, :].broadcast_to([P, 2, P]),
                    )
                exs.append((ex2, kt, colo))
            fillers = list(fillers)
            nf = len(fillers)
            for ex2, kt, colo in exs:
                for j in range(2):
                    h = 2 * i + j
                    nc.tensor.matmul(
                        accs[j][:, colo:SQ],
                        vv[:, kt, h * VW:(h + 1) * VW],
                        ex2[:, j, colo:SQ],
                        start=(kt == 0),
                        stop=(kt == nkt - 1),
                    )
                # dense PE filler between exp-paced attnv groups
                while fillers and len(fillers) > nf * (nkt - 1 - kt) // nkt:
                    fillers.pop(0)()
            dns = []
            for j in range(2):
                with nc.allow_low_precision(reason="fp16 matmul inputs"):
                    nc.vector.tensor_copy(
                        outT[i][j * 64:j * 64 + 64, c * SQ:(c + 1) * SQ],
                        accs[j][0:64, :],
                    )
                dn = dh_pool.tile([1, SQ], F16, name="dn", tag="dn")
                with nc.allow_low_precision(reason="fp16 matmul inputs"):
                    nc.vector.tensor_copy(dn, accs[j][64:65, :])
                dns.append(dn)
            return dns

        def norm_pair(c, i, dns):
            # outT *= 1/denominator: broadcast denoms via K=1 matmuls, one
            # 128-lane fast reciprocal, one fp16 multiply
            if True:
                db = psA.tile([P, SQ], F32, name="ps", tag="ps")
                nc.tensor.matmul(
                    db[0:64, :], ones1[:, 0:64], dns[0],
                    start=True, stop=True,
                )
                nc.tensor.matmul(
                    db[64:P, :], ones1[:, 0:64], dns[1],
                    start=True, stop=True,
                )
                rc32 = rc_pool.tile([P, SQ], F32, name="rc32", tag="rc32")
                nc.vector.reciprocal_approx_fast(rc32, db)
                rcpb = rc_pool.tile([P, SQ], F16, name="rcpb", tag="rcpb")
                with nc.allow_low_precision(reason="fp16 matmul inputs"):
                    nc.vector.tensor_copy(rcpb, rc32)
                nc.vector.tensor_mul(
                    outT[i][:, c * SQ:(c + 1) * SQ],
                    outT[i][:, c * SQ:(c + 1) * SQ],
                    rcpb,
                )

        def proj_mtile(m, only_kk=None, ps_list=None):
            for nch in range(2):
                if only_kk == 1:
                    ps = ps_list[nch]
                else:
                    ps = psA.tile([P, SQ], F32, name="ps", tag="ps")
                    if ps_list is not None:
                        ps_list.append(ps)
                kks = (0, 1) if only_kk is None else (only_kk,)
                for kk in kks:
                    nc.tensor.matmul(
                        ps,
                        outT[kk][:, m * P:(m + 1) * P],
                        wp_t[kk][:, nch * SQ:(nch + 1) * SQ],
                        start=(kk == 0),
                        stop=(kk == 1),
                    )
                if only_kk == 0:
                    continue
                ob = osb.tile([P, SQ], F16, name="ob", tag="ob")
                with nc.allow_low_precision(reason="partial sums; host sums fp32"):
                    nc.vector.tensor_copy(ob, ps)
                nc.sync.dma_start(
                    ap["out"][m * P:(m + 1) * P, nch * SQ:(nch + 1) * SQ], ob,
                )

        # ---- chunk-pipelined main body ----
        # chunk 0 QKV upfront, with warmup matmuls sprinkled between groups to
        # keep the PE (and its clock gate) busy while input DMAs stream in
        for gi, g in enumerate(qkv_groups(0)):
            g()
            for i in range(3):
                nc.tensor.matmul(
                    wps, wsrc[:, 0:P], wsrc,
                    start=(i == 0), stop=(i == 2),
                )
        # attention(c) runs against qkv chunks emitted one chunk ahead.
        # Dense PE filler between heads: remaining qkv chunks during c=1,2
        # and the saved-up projection tiles of chunks 0-2 during c=3 (the
        # largest, most exp-bound chunk). norm runs per head-pair so the
        # reciprocal chain starts as soon as both heads of a pair finish.
        for c in range(NSQ):
            nxt = list(qkv_groups(c + 1)) if c + 1 < NSQ else []
            for i in range(2):
                fillers = list(nxt[4 * i:4 * i + 4])
                if c == NSQ - 1:
                    fillers += [
                        (lambda m=m: proj_mtile(m))
                        for m in range(6 * i, 6 * i + 6)
                    ]
                dns = attention_pair(i, c, fillers)
                norm_pair(c, i, dns)
                if c == NSQ - 1 and i == 1:
                    # first contraction half of the last projection tiles can
                    # start as soon as outT[0] chunk 3 is normalized
                    tail_ps = {12: []}
                    proj_mtile(12, only_kk=0, ps_list=tail_ps[12])
        for m in range(4 * (NSQ - 1), 4 * NSQ):
            if m in tail_ps:
                proj_mtile(m, only_kk=1, ps_list=tail_ps[m])
            else:
                proj_mtile(m)


def build_program():
    nc = bacc.Bacc("TRN2", target_bir_lowering=False, debug=False,
                   num_devices=NCORES)
    ap = {}
    for name, shape, dt in (
        ("xT", [D, S], F16), ("wq", [D, CH], F16), ("wk", [D, CH], F16),
        ("wv", [D, HPC * VW], F16), ("bq", [P, 2], F32), ("bk", [P, 2], F32),
        ("bv", [1, HPC * VW], F16), ("wp", [CH, D], F16),
        ("tri", [P, P], F16), ("ones1", [1, P], F16),
    ):
        ap[name] = nc.dram_tensor(name, shape, dt, kind="ExternalInput").ap()
    ap["out"] = nc.dram_tensor("out", [S, D], F16, kind="ExternalOutput").ap()

    with tile.TileContext(nc) as tc:
        emit_kernel(nc, tc, ap)
    nc.compile()
    return nc


def make_core_inputs(hidden_states, w_attn, b_attn, w_proj):
    """Host-side sharding: per-core input dicts (core = batch*4 + head_group)."""
    f16, f32 = np.float16, np.float32
    x = np.asarray(hidden_states, f32)
    w_attn = np.asarray(w_attn, f32)
    b_attn = np.asarray(b_attn, f32)
    w_proj = np.asarray(w_proj, f32)

    tri = (np.arange(P)[:, None] <= np.arange(P)[None, :]).astype(f16)
    ones_row = np.ones((1, P), f16)
    xTs = [np.ascontiguousarray(x[b].T).astype(f16) for b in range(B)]

    in_maps = []
    for core in range(NCORES):
        b, g = core // HPC, core % HPC
        wq = np.ascontiguousarray(w_attn[:, g * CH:(g + 1) * CH]).astype(f16)
        wk = np.ascontiguousarray(
            w_attn[:, D + g * CH:D + (g + 1) * CH]).astype(f16)
        wv = np.zeros((D, HPC * VW), f16)
        bv = np.zeros((1, HPC * VW), f16)
        for h in range(HPC):
            src = 2 * D + (g * HPC + h) * HD
            wv[:, h * VW:h * VW + HD] = w_attn[:, src:src + HD]
            bv[0, h * VW:h * VW + HD] = b_attn[src:src + HD]
            bv[0, h * VW + HD] = 1.0
        bq = np.ascontiguousarray(
            b_attn[g * CH:(g + 1) * CH].reshape(2, P).T)
        bk = np.ascontiguousarray(
            b_attn[D + g * CH:D + (g + 1) * CH].reshape(2, P).T)
        wp = np.ascontiguousarray(w_proj[g * CH:(g + 1) * CH, :]).astype(f16)
        in_maps.append({
            "xT": xTs[b], "wq": wq, "wk": wk, "wv": wv,
            "bq": bq, "bk": bk, "bv": bv, "wp": wp,
            "tri": tri, "ones1": ones_row,
        })
    return in_maps


_PROGRAM = None


def kernel(hidden_states, w_attn, b_attn, w_proj, b_proj):
    global _PROGRAM
    if _PROGRAM is None:
        _PROGRAM = build_program()
    in_maps = make_core_inputs(hidden_states, w_attn, b_attn, w_proj)
    res = run_bass_kernel_spmd(_PROGRAM, in_maps, core_ids=list(range(NCORES)))
    out = np.zeros((B, S, D), np.float32)
    for core in range(NCORES):
        out[core // HPC] += res.results[core]["out"].astype(np.float32)
    out += np.asarray(b_proj, np.float32)
    return out


# revision 27
# speedup vs baseline: 1.0313x; 1.0313x over previous
"""Fused causal multi-head attention block on 8 Trainium2 NeuronCores.

Problem (GPT-2 style attention, B=2, S=2048, D=1024, H=16, hd=64):
    qkv = x @ w_attn + b_attn ; split q,k,v ; per-head causal softmax(q k^T / 8) v
    out = attn_out @ w_proj + b_proj

Sharding: data parallel on batch (2) x tensor parallel on heads (4 groups of 4
heads). Core c -> batch c//4, head group c%4. Each core computes a partial
[S, D] output (its heads' slice of w_proj rows); host sums the 4 partials per
batch and adds b_proj.

Per-core kernel layout tricks:
- scores are computed TRANSPOSED (scoresT[key, query]) so the softmax
  denominator falls out of the attn@v matmul by appending a ones-column to v:
  [v | 1]^T @ exp(scoresT) yields the unnormalized output and the per-query
  denominator in one PSUM accumulation.
- matmul inputs are fp16 (full PE rate + fast weight loads); all accumulation
  is fp32 in PSUM. exp(s/8) is in [0, ~13], well inside fp16 range.
- causal masking: fully-masked blocks are skipped via restricted matmul
  widths; diagonal blocks get a -30000 triangle accumulated into the score
  PSUM by an identity matmul, so exp() yields exact zeros and the vector
  engine stays out of the score->attnv chain.
- emission is chunk-pipelined (QKV chunk c, attention chunk c, projection
  chunk c) so the PE always has dense matmul work while ScalarE runs exp.
"""

import sys

sys.path.insert(0, "/opt/trn_rl_repo")

import numpy as np

import concourse.bass as bass
import concourse.mybir as mybir
import concourse.tile as tile
from concourse import bacc
from concourse.bass_utils import run_bass_kernel_spmd

F32 = mybir.dt.float32
F16 = mybir.dt.float16
AFT = mybir.ActivationFunctionType

B, S, D, H, HD = 2, 2048, 1024, 16, 64
NCORES = 8
HPC = 4            # heads per core
CH = HPC * HD      # 256 channels per core
VW = HD + 1        # v width incl. ones column
P = 128
KT = D // P        # 8 contraction tiles over D
SQ = 512           # query/N chunk
NSQ = S // SQ      # 4
NST = S // P       # 16 seq tiles
SCALE = 1.0 / np.sqrt(HD)
MASKNEG = -30000.0


def emit_kernel(nc, tc, ap):
    """Emit the per-core program. `ap` is a dict of DRAM APs."""
    with (
        tc.tile_pool(name="const", bufs=1) as cp,
        tc.tile_pool(name="xw", bufs=1) as xw,
        tc.tile_pool(name="act", bufs=1) as acts,
        tc.tile_pool(name="ex", bufs=16) as exp_pool,
        tc.tile_pool(name="dh", bufs=4) as dh_pool,
        tc.tile_pool(name="rc", bufs=2) as rc_pool,
        tc.tile_pool(name="osb", bufs=3) as osb,
        tc.tile_pool(name="psA", bufs=2, space="PSUM") as psA,
        tc.tile_pool(name="psB", bufs=2, space="PSUM") as psB,
        tc.tile_pool(name="psC", bufs=2, space="PSUM") as psC,
    ):
        # ---- PE warmup: dense dummy matmuls while input DMAs stream in.
        # The PE clock-gate (HAM) unthrottles 1.2->2.4 GHz only after ~3.4us
        # of sustained matmul activity; burn that in on scratch data.
        wsrc = cp.tile([P, SQ], F16, name="wsrc", tag="wsrc")
        nc.gpsimd.memset(wsrc, 0.0)
        wps = psB.tile([P, SQ], F32, name="wps", tag="acc")
        for i in range(16):
            nc.tensor.matmul(
                wps, wsrc[:, 0:P], wsrc, start=(i == 0), stop=(i == 15),
            )

        # ---- constants ----
        tri = cp.tile([P, P], F16, name="tri", tag="tri")
        nc.sync.dma_start(tri, ap["tri"])
        bq = cp.tile([P, 2], F32, name="bq", tag="bq")
        nc.sync.dma_start(bq, ap["bq"])
        bk = cp.tile([P, 2], F32, name="bk", tag="bk")
        nc.sync.dma_start(bk, ap["bk"])
        bv = cp.tile([1, HPC * VW], F16, name="bv", tag="bv")
        nc.sync.dma_start(bv, ap["bv"])
        ones1 = cp.tile([1, P], F16, name="ones1", tag="ones1")
        nc.sync.dma_start(ones1, ap["ones1"])

        # ---- weight/x loads. Few big DMAs: each dma_start costs ~600ns of
        # serialized issue on the Sync queue, so 34 small loads would stagger
        # the late k-tiles by ~20us. Two halves for x (so the QKV k-loop can
        # start on the first half), one DMA per weight tensor.
        def kmaj(dram_ap, rows, cols):
            return dram_ap[0:rows, :].rearrange("(k p) c -> p k c", p=P)
        xts = xw.tile([P, KT, S], F16, name="xts", tag="xts")
        half = KT // 2
        nc.sync.dma_start(xts[:, 0:half, :], kmaj(ap["xT"], half * P, S))
        nc.sync.dma_start(
            xts[:, half:KT, :],
            ap["xT"][half * P:KT * P, :].rearrange("(k p) c -> p k c", p=P),
        )
        wq = xw.tile([P, KT, CH], F16, name="wq", tag="wq")
        nc.sync.dma_start(wq, kmaj(ap["wq"], KT * P, CH))
        wk = xw.tile([P, KT, CH], F16, name="wk", tag="wk")
        nc.sync.dma_start(wk, kmaj(ap["wk"], KT * P, CH))
        wv = xw.tile([P, KT, HPC * VW], F16, name="wv", tag="wv")
        nc.sync.dma_start(wv, kmaj(ap["wv"], KT * P, HPC * VW))
        wp = xw.tile([P, 2, D], F16, name="wp", tag="wp")
        nc.sync.dma_start(wp, kmaj(ap["wp"], 2 * P, D))
        xts_k = [xts[:, k, :] for k in range(KT)]
        wq_t = [wq[:, k, :] for k in range(KT)]
        wk_t = [wk[:, k, :] for k in range(KT)]
        wv_t = [wv[:, k, :] for k in range(KT)]
        wp_t = [wp[:, k, :] for k in range(2)]

        # ---- activations living across phases ----
        qT = [acts.tile([P, S], F16, name=f"qT{i}", tag=f"qT{i}") for i in range(2)]
        kTt = [acts.tile([P, S], F16, name=f"kT{i}", tag=f"kT{i}") for i in range(2)]
        vv = acts.tile([P, NST, HPC * VW], F16, name="vv", tag="vv")
        outT = [acts.tile([P, S], F16, name=f"oT{i}", tag=f"oT{i}") for i in range(2)]

        def qkv_qk_group(c, dst, wt, bias, i):
            ps = psA.tile([P, SQ], F32, name="ps", tag="ps")
            for k in range(KT):
                nc.tensor.matmul(
                    ps,
                    wt[k][:, i * P:(i + 1) * P],
                    xts_k[k][:, c * SQ:(c + 1) * SQ],
                    start=(k == 0),
                    stop=(k == KT - 1),
                )
            with nc.allow_low_precision(reason="fp16 matmul inputs"):
                nc.vector.tensor_scalar_add(
                    dst[i][:, c * SQ:(c + 1) * SQ], ps, bias[:, i:i + 1],
                )

        def qkv_v_group(st):
            # v rows (natural layout + interleaved ones cols)
            ps = psA.tile([P, SQ], F32, name="psv", tag="ps")
            psv = ps[:, 0:HPC * VW]
            for k in range(KT):
                nc.tensor.matmul(
                    psv,
                    xts_k[k][:, st * P:(st + 1) * P],
                    wv_t[k],
                    start=(k == 0),
                    stop=False,
                )
            # += ones_col(seq) x (bv | interleaved 1.0): v-bias + ones col
            nc.tensor.matmul(psv, ones1, bv, start=False, stop=True)
            with nc.allow_low_precision(reason="fp16 matmul inputs"):
                nc.vector.tensor_copy(vv[:, st, :], psv)

        def qkv_groups(c):
            for dst, wt, bias in ((qT, wq_t, bq), (kTt, wk_t, bk)):
                for i in range(2):
                    yield lambda dst=dst, wt=wt, bias=bias, i=i: \
                        qkv_qk_group(c, dst, wt, bias, i)
            for st in range(4 * c, 4 * c + 4):
                yield lambda st=st: qkv_v_group(st)

        def attention_pair(i, c, fillers=()):
            """Heads 2i (kT/qT rows 0:64) and 2i+1 (rows 64:128) together.

            Both heads' scores for a key tile land in one 2-bank PSUM tile so
            a single exp instruction covers them (halves ScalarE instruction
            count). All scores are emitted before all attnv matmuls: the PE
            stream is in-order, so this keeps the PE on scores while
            ScalarE's exps pipeline behind."""
            nkt = 4 * (c + 1)
            accs = [psB.tile([VW, SQ], F32, name="acc", tag="acc")
                    for _ in range(2)]

            exs = []
            for kt in range(nkt):
                colo = max(0, kt * P - c * SQ)
                diag = colo > 0 or kt * P == c * SQ
                sc2 = psC.tile([P, 2, SQ], F32, name="sc2", tag="sc")
                for j in range(2):
                    ro = j * 64
                    nc.tensor.matmul(
                        sc2[:, j, colo:SQ],
                        kTt[i][ro:ro + 64, kt * P:(kt + 1) * P],
                        qT[i][ro:ro + 64, c * SQ + colo:(c + 1) * SQ],
                        start=True,
                        stop=True,
                    )
                ex2 = exp_pool.tile([P, 2, SQ], F16, name="ex2", tag="ex")
                nc.scalar.activation(
                    ex2[:, :, colo:SQ], sc2[:, :, colo:SQ], AFT.Exp, scale=SCALE,
                )
                if diag:
                    # zero the masked triangle of the diagonal block; runs on
                    # VectorE well before the (much later) attnv consumers
                    nc.vector.tensor_mul(
                        ex2[:, :, colo:colo + P],
                        ex2[:, :, colo:colo + P],
                        tri[:, None, :].broadcast_to([P, 2, P]),
                    )
                exs.append((ex2, kt, colo))
            fillers = list(fillers)
            nf = len(fillers)
            for ex2, kt, colo in exs:
                for j in range(2):
                    h = 2 * i + j
                    nc.tensor.matmul(
                        accs[j][:, colo:SQ],
                        vv[:, kt, h * VW:(h + 1) * VW],
                        ex2[:, j, colo:SQ],
                        start=(kt == 0),
                        stop=(kt == nkt - 1),
                    )
                # dense PE filler between exp-paced attnv groups
                while fillers and len(fillers) > nf * (nkt - 1 - kt) // nkt:
                    fillers.pop(0)()
            dns = []
            for j in range(2):
                with nc.allow_low_precision(reason="fp16 matmul inputs"):
                    nc.vector.tensor_copy(
                        outT[i][j * 64:j * 64 + 64, c * SQ:(c + 1) * SQ],
                        accs[j][0:64, :],
                    )
                dn = dh_pool.tile([1, SQ], F16, name="dn", tag="dn")
                with nc.allow_low_precision(reason="fp16 matmul inputs"):
                    nc.vector.tensor_copy(dn, accs[j][64:65, :])
                dns.append(dn)
            return dns

        def norm_pair(c, i, dns):
            # outT *= 1/denominator: broadcast denoms via K=1 matmuls, one
            # 128-lane fast reciprocal, one fp16 multiply
            if True:
                db = psA.tile([P, SQ], F32, name="ps", tag="ps")
                nc.tensor.matmul(
                    db[0:64, :], ones1[:, 0:64], dns[0],
                    start=True, stop=True,
                )
                nc.tensor.matmul(
                    db[64:P, :], ones1[:, 0:64], dns[1],
                    start=True, stop=True,
                )
                rc32 = rc_pool.tile([P, SQ], F32, name="rc32", tag="rc32")
                nc.vector.reciprocal_approx_fast(rc32, db)
                rcpb = rc_pool.tile([P, SQ], F16, name="rcpb", tag="rcpb")
                with nc.allow_low_precision(reason="fp16 matmul inputs"):
                    nc.vector.tensor_copy(rcpb, rc32)
                nc.vector.tensor_mul(
                    outT[i][:, c * SQ:(c + 1) * SQ],
                    outT[i][:, c * SQ:(c + 1) * SQ],
                    rcpb,
                )

        def proj_mtile(m, only_kk=None, ps_list=None):
            for nch in range(2):
                if only_kk == 1:
                    ps = ps_list[nch]
                else:
                    ps = psA.tile([P, SQ], F32, name="ps", tag="ps")
                    if ps_list is not None:
                        ps_list.append(ps)
                kks = (0, 1) if only_kk is None else (only_kk,)
                for kk in kks:
                    nc.tensor.matmul(
                        ps,
                        outT[kk][:, m * P:(m + 1) * P],
                        wp_t[kk][:, nch * SQ:(nch + 1) * SQ],
                        start=(kk == 0),
                        stop=(kk == 1),
                    )
                if only_kk == 0:
                    continue
                ob = osb.tile([P, SQ], F16, name="ob", tag="ob")
                with nc.allow_low_precision(reason="partial sums; host sums fp32"):
                    nc.vector.tensor_copy(ob, ps)
                nc.sync.dma_start(
                    ap["out"][m * P:(m + 1) * P, nch * SQ:(nch + 1) * SQ], ob,
                )

        # ---- chunk-pipelined main body ----
        # chunk 0 QKV upfront, with warmup matmuls sprinkled between groups to
        # keep the PE (and its clock gate) busy while input DMAs stream in
        for gi, g in enumerate(qkv_groups(0)):
            g()
            for i in range(3):
                nc.tensor.matmul(
                    wps, wsrc[:, 0:P], wsrc,
                    start=(i == 0), stop=(i == 2),
                )
        # attention(c) runs against qkv chunks emitted one chunk ahead.
        # Dense PE filler between heads: remaining qkv chunks during c=1,2
        # and the saved-up projection tiles of chunks 0-2 during c=3 (the
        # largest, most exp-bound chunk). norm runs per head-pair so the
        # reciprocal chain starts as soon as both heads of a pair finish.
        for c in range(NSQ):
            nxt = list(qkv_groups(c + 1)) if c + 1 < NSQ else []
            for i in range(2):
                fillers = list(nxt[4 * i:4 * i + 4])
                if c == NSQ - 1:
                    fillers += [
                        (lambda m=m: proj_mtile(m))
                        for m in range(6 * i, 6 * i + 6)
                    ]
                dns = attention_pair(i, c, fillers)
                norm_pair(c, i, dns)
                if c == NSQ - 1 and i == 1:
                    # first contraction half of the last projection tiles can
                    # start as soon as outT[0] chunk 3 is normalized
                    tail_ps = {12: []}
                    proj_mtile(12, only_kk=0, ps_list=tail_ps[12])
        for m in range(4 * (NSQ - 1), 4 * NSQ):
            if m in tail_ps:
                proj_mtile(m, only_kk=1, ps_list=tail_ps[m])
            else:
                proj_mtile(m)


def build_program():
    nc = bacc.Bacc("TRN2", target_bir_lowering=False, debug=False,
                   num_devices=NCORES)
    ap = {}
    for name, shape, dt in (
        ("xT", [D, S], F16), ("wq", [D, CH], F16), ("wk", [D, CH], F16),
        ("wv", [D, HPC * VW], F16), ("bq", [P, 2], F32), ("bk", [P, 2], F32),
        ("bv", [1, HPC * VW], F16), ("wp", [CH, D], F16),
        ("tri", [P, P], F16), ("ones1", [1, P], F16),
    ):
        ap[name] = nc.dram_tensor(name, shape, dt, kind="ExternalInput").ap()
    ap["out"] = nc.dram_tensor("out", [S, D], F16, kind="ExternalOutput").ap()

    with tile.TileContext(nc) as tc:
        emit_kernel(nc, tc, ap)
    nc.compile()
    return nc


def make_core_inputs(hidden_states, w_attn, b_attn, w_proj):
    """Host-side sharding: per-core input dicts (core = batch*4 + head_group)."""
    f16, f32 = np.float16, np.float32
    x = np.asarray(hidden_states, f32)
    w_attn = np.asarray(w_attn, f32)
    b_attn = np.asarray(b_attn, f32)
    w_proj = np.asarray(w_proj, f32)

    tri = (np.arange(P)[:, None] <= np.arange(P)[None, :]).astype(f16)
    ones_row = np.ones((1, P), f16)
    xTs = [np.ascontiguousarray(x[b].T).astype(f16) for b in range(B)]

    in_maps = []
    for core in range(NCORES):
        b, g = core // HPC, core % HPC
        wq = np.ascontiguousarray(w_attn[:, g * CH:(g + 1) * CH]).astype(f16)
        wk = np.ascontiguousarray(
            w_attn[:, D + g * CH:D + (g + 1) * CH]).astype(f16)
        wv = np.zeros((D, HPC * VW), f16)
        bv = np.zeros((1, HPC * VW), f16)
        for h in range(HPC):
            src = 2 * D + (g * HPC + h) * HD
            wv[:, h * VW:h * VW + HD] = w_attn[:, src:src + HD]
            bv[0, h * VW:h * VW + HD] = b_attn[src:src + HD]
            bv[0, h * VW + HD] = 1.0
        bq = np.ascontiguousarray(
            b_attn[g * CH:(g + 1) * CH].reshape(2, P).T)
        bk = np.ascontiguousarray(
            b_attn[D + g * CH:D + (g + 1) * CH].reshape(2, P).T)
        wp = np.ascontiguousarray(w_proj[g * CH:(g + 1) * CH, :]).astype(f16)
        in_maps.append({
            "xT": xTs[b], "wq": wq, "wk": wk, "wv": wv,
            "bq": bq, "bk": bk, "bv": bv, "wp": wp,
            "tri": tri, "ones1": ones_row,
        })
    return in_maps


_PROGRAM = None


def kernel(hidden_states, w_attn, b_attn, w_proj, b_proj):
    global _PROGRAM
    if _PROGRAM is None:
        _PROGRAM = build_program()
    in_maps = make_core_inputs(hidden_states, w_attn, b_attn, w_proj)
    res = run_bass_kernel_spmd(_PROGRAM, in_maps, core_ids=list(range(NCORES)))
    out = np.zeros((B, S, D), np.float32)
    for core in range(NCORES):
        out[core // HPC] += res.results[core]["out"].astype(np.float32)
    out += np.asarray(b_proj, np.float32)
    return out
